# revision 10
# baseline (speedup 1.0000x reference)
"""Trainium2 Bass kernel for nn_AlgebraicFusion (complex bank mixer).

Algebra (per row l, bank n):
  y_n = P_n x_n            P_n = phase-folded bank_W (host precompute)
  w   = softmax(MLP(mean|x_n|^2))
  c   = sum_n w_n y_n
  out = O' (gamma * c * rsqrt(mean|c|^2 + eps))

Device computation (per core, 1024 rows, f16 matmul inputs / f32 accum):
  - mag_n = sum_d |x_n|^2 via ACT Square+accum on unscaled f16 casts
  - router MLP on PE (transposed [*, l] layout); softmax division is
    deferred: unnormalized e_n scale x, and the denominator s folds into
    the final rsqrt:  c*rsqrt(mean|c/s|^2+eps) = chat*rsqrt(SS/D+eps*s^2)
  - x'_c = e_n * x_c  (DVE f16), PE-transpose into x'^T combos [i, l]
  - Karatsuba complex matmul, c^T accumulated in PSUM over (bank, i-chunk):
      A = sum Pr^T xs', B = sum (Pi-Pr)^T xr', C = sum (Pr+Pi)^T xi'
      cr^T = A-C, ci^T = A+B   [o, l] f16
  - SS = sum_o c^2 via ones-matmul (partition reduce), inv = rsqrt chain
  - final Karatsuba matmul vs out_W combos (gamma folded); contraction o
    is already c^T's partition dim (no transpose); scale by inv per row,
    interleave (r,i), DMA out.

Sharding: rows (B*L = 8192) split evenly across 8 cores; weights replicated.
"""
import numpy as np

NB, B, L, D = 4, 4, 2048, 1024
EPS, NORM_EPS = 1e-8, 1e-5
NCORES = 8
ROWS = B * L // NCORES          # 1024 rows per core
LT = ROWS // 128                # 8 l-tiles
BLK = 2                         # l-tiles per block
NBLK = LT // BLK                # 4 blocks
BN = BLK * 128                  # 256 rows per block
IC = 8                          # i-chunks (contraction 1024 = 8*128)
OT = 8                          # o-tiles
DH = 2                          # d2 halves of 512


def build_program(gelu_exact=True):
    import concourse.bacc as bacc
    import concourse.tile as tile
    import concourse.mybir as mybir

    AF = mybir.ActivationFunctionType
    from concourse.alu_op_type import AluOpType

    f16 = mybir.dt.float16
    f32 = mybir.dt.float32
    f8 = mybir.dt.float8e4
    GELU = AF.Gelu if gelu_exact else AF.Tanh

    nc = bacc.Bacc("TRN2", target_bir_lowering=False, debug=False,
                   num_devices=NCORES)

    xin = nc.dram_tensor("xin", [NB, ROWS, 2 * D], f32, kind="ExternalInput").ap()
    pk = nc.dram_tensor("pk", [OT, NB, 3, 128, 1024], f16,
                        kind="ExternalInput").ap()
    okk = nc.dram_tensor("okk", [DH, 128, 3 * OT * 512], f16,
                         kind="ExternalInput").ap()
    w1t = nc.dram_tensor("w1t", [4, 16], f32, kind="ExternalInput").ap()
    b1c = nc.dram_tensor("b1c", [16, 1], f32, kind="ExternalInput").ap()
    w2t = nc.dram_tensor("w2t", [16, 4], f32, kind="ExternalInput").ap()
    b2c = nc.dram_tensor("b2c", [4, 1], f32, kind="ExternalInput").ap()
    eye16d = nc.dram_tensor("eye16", [128, 128], f16, kind="ExternalInput").ap()
    eye32d = nc.dram_tensor("eye32", [128, 128], f32, kind="ExternalInput").ap()
    outd = nc.dram_tensor("out", [ROWS, 2 * D], f32, kind="ExternalOutput").ap()

    with tile.TileContext(nc) as tc:
        import contextlib
        with contextlib.ExitStack() as ctx:
            cst = ctx.enter_context(tc.tile_pool(name="cst", bufs=1))
            wk = ctx.enter_context(tc.tile_pool(name="wk", bufs=1))
            psacc = ctx.enter_context(tc.tile_pool(name="psacc", bufs=6, space="PSUM"))
            pssm = ctx.enter_context(tc.tile_pool(name="pssm", bufs=2, space="PSUM"))

            # ---- constants ----
            eye16 = cst.tile([128, 128], f16)
            nc.sync.dma_start(eye16[:], eye16d)
            eye32 = cst.tile([128, 128], f32)
            nc.sync.dma_start(eye32[:], eye32d)
            w1s = cst.tile([4, 16], f32)
            nc.sync.dma_start(w1s[:], w1t)
            b1s = cst.tile([16, 1], f32)
            nc.sync.dma_start(b1s[:], b1c)
            w2s = cst.tile([16, 4], f32)
            nc.sync.dma_start(w2s[:], w2t)
            b2s = cst.tile([4, 1], f32)
            nc.sync.dma_start(b2s[:], b2c)
            onesD = cst.tile([128, 1], f16)      # 1/D for the SS ones-matmul
            nc.vector.memset(onesD[:], 1.0 / D)
            ones4 = cst.tile([4, 1], f32)
            nc.vector.memset(ones4[:], 1.0)
            sseps = cst.tile([1, ROWS], f32)     # s^2 per row
            invrow = cst.tile([1, ROWS], f32)    # rsqrt result per row

            # OK combos resident: [128 ow, (dh,c,oc)*512] f16, 2 big DMAs
            oks = cst.tile([128, DH * 3 * OT * 512], f16)
            for dh in range(DH):
                nc.gpsimd.dma_start(oks[:, dh * 12288:(dh + 1) * 12288], okk[dh])

            # XT: x'^T combos, one tile per (n, c): [128 iw, IC*BN] f16
            # c order: 0=s, 1=r, 2=i
            xt = {(n, c): cst.tile([128, IC * BN], f16, name=f"xt{n}{c}")
                  for n in range(NB) for c in range(3)}
            # CT: c^T combos, one tile per (c, ot): [128 ow, BN] f16
            # c order: 0=r, 1=i, 2=s
            ct = {(c, o): cst.tile([128, BN], f16, name=f"ct{c}{o}")
                  for c in range(3) for o in range(OT)}

            for blk in range(NBLK):
                t0 = blk * BLK
                # ---- phase 1+2 per l-tile: load, cast, mag, router, scale,
                #      transpose into XT ----
                for tb in range(BLK):
                    t = t0 + tb
                    xu = {}
                    raws = []
                    for n in range(NB):
                        raw = wk.tile([128, 2 * D], f32, tag="raw", bufs=2)
                        nc.gpsimd.dma_start(raw[:], xin[n, t * 128:(t + 1) * 128, :])
                        raws.append(raw)
                    # interleaved f16 casts (one Copy per bank, contiguous read)
                    for n in range(NB):
                        xc = wk.tile([128, 2 * D], f16, tag="xu", bufs=5)
                        nc.scalar.copy(xc[:], raws[n][:])
                        xu[n] = xc
                    # squares: accum over interleaved row = sum(xr^2+xi^2)
                    mag = wk.tile([128, NB], f32, tag="mag", bufs=2)
                    for n in range(NB):
                        junk = wk.tile([128, 2 * D], f8, tag="junk", bufs=1)
                        nc.scalar.activation(junk[:], xu[n][:], AF.Square,
                                             accum_out=mag[:, n:n + 1])

                    # router (transposed [*, l] layout)
                    magTp = pssm.tile([NB, 128], f32, tag="sm")
                    nc.tensor.transpose(magTp[:], mag[:], eye32[:])
                    magT = wk.tile([NB, 128], f32, tag="magT", bufs=2)
                    nc.vector.tensor_copy(magT[:], magTp[:])
                    h1p = pssm.tile([16, 128], f32, tag="sm")
                    nc.tensor.matmul(h1p[:], w1s[:], magT[:], start=True, stop=True)
                    h1 = wk.tile([16, 128], f32, tag="h1", bufs=2)
                    nc.scalar.activation(h1[:], h1p[:], GELU, bias=b1s[:])
                    lgp = pssm.tile([4, 128], f32, tag="sm")
                    nc.tensor.matmul(lgp[:], w2s[:], h1[:], start=True, stop=True)
                    e4 = wk.tile([4, 128], f32, tag="e4", bufs=2)
                    nc.scalar.activation(e4[:], lgp[:], AF.Exp, bias=b2s[:])
                    ssump = pssm.tile([1, 128], f32, tag="sm")
                    nc.tensor.matmul(ssump[:], ones4[:], e4[:], start=True,
                                     stop=True)
                    s_sb = wk.tile([1, 128], f32, tag="s_sb", bufs=2)
                    nc.vector.tensor_copy(s_sb[:], ssump[:])
                    nc.vector.tensor_mul(sseps[0:1, t * 128:(t + 1) * 128],
                                         s_sb[:], s_sb[:])
                    eTp = pssm.tile([128, NB], f32, tag="sm")
                    nc.tensor.transpose(eTp[:], e4[:], eye32[0:4, 0:4])
                    wsc = wk.tile([128, NB], f32, tag="wsc", bufs=2)
                    nc.vector.tensor_copy(wsc[:], eTp[:])

                    # scale (interleaved) + strided transposes into XT
                    for n in range(NB):
                        xs = wk.tile([128, 2 * D], f16, tag="xsc", bufs=2)
                        nc.vector.tensor_scalar_mul(xs[:], xu[n][:],
                                                    wsc[:, n:n + 1])
                        xsv = xs[:].rearrange("p (d c) -> p c d", c=2)
                        for comp in (0, 1):   # 0=r -> XT c=1, 1=i -> XT c=2
                            dstt = xt[(n, 1 + comp)]
                            for icc in range(IC):
                                trp = pssm.tile([128, 128], f16, tag="sm")
                                nc.tensor.transpose(
                                    trp[:],
                                    xsv[:, comp, icc * 128:(icc + 1) * 128],
                                    eye16[:])
                                dst = icc * BN + tb * 128
                                nc.vector.tensor_copy(dstt[:, dst:dst + 128],
                                                      trp[:])
                for n in range(NB):
                    nc.vector.tensor_add(xt[(n, 0)][:], xt[(n, 1)][:],
                                         xt[(n, 2)][:])

                # ---- phase 3: bank matmuls -> c^T (ot-major, PK streamed) ----
                for ot_ in range(OT):
                    pkt = {}
                    for c in range(3):
                        for n in range(NB):
                            pt = wk.tile([128, 1024], f16, tag="pkt", bufs=7,
                                         name=f"pkt{blk}_{ot_}_{c}_{n}")
                            nc.sync.dma_start(pt[:], pk[ot_, n, c])
                            pkt[(n, c)] = pt
                    psA = psacc.tile([128, BN], f32, tag="acc")
                    psB = psacc.tile([128, BN], f32, tag="acc")
                    psC = psacc.tile([128, BN], f32, tag="acc")
                    for c, ps in ((0, psA), (1, psB), (2, psC)):
                        first = True
                        for n in range(NB):
                            for icc in range(IC):
                                nc.tensor.matmul(
                                    ps[:], pkt[(n, c)][:, icc * 128:(icc + 1) * 128],
                                    xt[(n, c)][:, icc * BN:(icc + 1) * BN],
                                    start=first,
                                    stop=(n == NB - 1 and icc == IC - 1))
                                first = False
                    aA = wk.tile([128, BN], f32, tag="aA", bufs=2)
                    nc.vector.tensor_copy(aA[:], psA[:])
                    nc.vector.tensor_sub(ct[(0, ot_)][:], aA[:], psC[:])
                    nc.vector.tensor_add(ct[(1, ot_)][:], aA[:], psB[:])
                    nc.vector.tensor_add(ct[(2, ot_)][:], ct[(0, ot_)][:],
                                         ct[(1, ot_)][:])

                # ---- phase 4: SS (partition reduce via ones-matmul) ----
                ssp = pssm.tile([1, BN], f32, tag="sm")
                first = True
                for ot_ in range(OT):
                    for c in (0, 1):
                        c2 = wk.tile([128, BN], f16, tag="c2", bufs=2)
                        nc.scalar.activation(c2[:], ct[(c, ot_)][:], AF.Square)
                        nc.tensor.matmul(ssp[:], onesD[:], c2[:],
                                         start=first,
                                         stop=(ot_ == OT - 1 and c == 1))
                        first = False
                sms = wk.tile([1, BN], f32, tag="sms", bufs=2)
                nc.vector.scalar_tensor_tensor(
                    sms[:], sseps[0:1, t0 * 128:t0 * 128 + BN], NORM_EPS,
                    ssp[:], AluOpType.mult, AluOpType.add)
                rec = wk.tile([1, BN], f32, tag="rec", bufs=2)
                nc.vector.reciprocal(rec[:], sms[:])
                nc.scalar.activation(invrow[0:1, t0 * 128:t0 * 128 + BN],
                                     rec[:], AF.Sqrt)

                # ---- phase 5: final matmuls + scale + interleave + out ----
                for tb in range(BLK):
                    t = t0 + tb
                    invp = pssm.tile([128, 1], f32, tag="sm")
                    nc.tensor.transpose(invp[:],
                                        invrow[0:1, t * 128:(t + 1) * 128],
                                        eye32[0:1, 0:1])
                    invc = wk.tile([128, 1], f32, tag="invc", bufs=2)
                    nc.vector.tensor_copy(invc[:], invp[:])
                    for dh in range(DH):
                        fA = psacc.tile([128, 512], f32, tag="acc")
                        fB = psacc.tile([128, 512], f32, tag="acc")
                        fC = psacc.tile([128, 512], f32, tag="acc")
                        # A' <- cs x OK-A ; B' <- cr x OK-B ; C' <- ci x OK-C
                        for ps, ctc, okc in ((fA, 2, 0), (fB, 0, 1), (fC, 1, 2)):
                            for oc in range(OT):
                                ro = ((dh * 3 + okc) * OT + oc) * 512
                                nc.tensor.matmul(
                                    ps[:],
                                    ct[(ctc, oc)][:, tb * 128:tb * 128 + 128],
                                    oks[:, ro:ro + 512],
                                    start=(oc == 0), stop=(oc == OT - 1))
                        fa = wk.tile([128, 512], f32, tag="fa", bufs=1)
                        nc.vector.tensor_copy(fa[:], fA[:])
                        fr = wk.tile([128, 512], f32, tag="fr", bufs=2)
                        nc.vector.tensor_sub(fr[:], fa[:], fC[:])
                        fi = wk.tile([128, 512], f32, tag="fi", bufs=2)
                        nc.vector.tensor_add(fi[:], fa[:], fB[:])
                        oto = wk.tile([128, 1024], f32, tag="oto", bufs=2)
                        ov = oto[:].rearrange("p (d c) -> p c d", c=2)
                        nc.scalar.activation(ov[:, 0], fr[:], AF.Copy,
                                             scale=invc[:])
                        nc.scalar.activation(ov[:, 1], fi[:], AF.Copy,
                                             scale=invc[:])
                        nc.gpsimd.dma_start(
                            outd[t * 128:(t + 1) * 128,
                                 dh * 1024:(dh + 1) * 1024], oto[:])

    nc.compile()
    return nc


def host_prep(inputs):
    """Build per-core in_maps from full inputs (numpy f32)."""
    f16 = np.float16
    phase = np.asarray(inputs["phase"], np.float32)
    bank_W = np.asarray(inputs["bank_W"], np.float32)
    W1 = np.asarray(inputs["W1"], np.float32)
    b1 = np.asarray(inputs["b1"], np.float32)
    W2 = np.asarray(inputs["W2"], np.float32)
    b2 = np.asarray(inputs["b2"], np.float32)
    gamma = np.asarray(inputs["gamma"], np.float32)
    out_W = np.asarray(inputs["out_W"], np.float32)
    bank_out = np.asarray(inputs["bank_out"], np.float32)

    pr, pi = phase[..., 0], phase[..., 1]
    pm = np.sqrt(pr * pr + pi * pi) + EPS
    ur, ui = (pr / pm)[:, :, None], (pi / pm)[:, :, None]
    Wr, Wi = bank_W[..., 0], bank_W[..., 1]
    Pr = Wr * ur - Wi * ui
    Pi_ = Wr * ui + Wi * ur
    KT = np.stack([Pr, Pi_ - Pr, Pr + Pi_], 1).transpose(0, 1, 3, 2)  # [n,c,i,o]
    # pk[ot, n, c, iw, ic*128+ow] = KT[n, c, ic*128+iw, ot*128+ow]
    pkarr = np.ascontiguousarray(
        KT.reshape(NB, 3, IC, 128, OT, 128).transpose(4, 0, 1, 3, 2, 5)
        .reshape(OT, NB, 3, 128, 1024).astype(f16))

    Og = out_W * gamma[None, :, None]          # scale c-dim (col index)
    Or, Oi = Og[..., 0], Og[..., 1]
    OKT = np.stack([Or, Oi - Or, Or + Oi], 0).transpose(0, 2, 1)  # [c, i, d2]
    # okk[dh, ow, (c*8+oc)*512+d2w] = OKT[c, oc*128+ow, dh*512+d2w]
    okarr = np.ascontiguousarray(
        OKT.reshape(3, OT, 128, DH, 512).transpose(3, 2, 0, 1, 4)
        .reshape(DH, 128, 3 * OT * 512).astype(f16))

    w1tb = np.ascontiguousarray((W1 / D).T.astype(np.float32))      # [4, 16]
    b1cb = np.ascontiguousarray(b1[:, None].astype(np.float32))     # [16, 1]
    w2tb = np.ascontiguousarray(W2.T.astype(np.float32))            # [16, 4]
    b2cb = np.ascontiguousarray(b2[:, None].astype(np.float32))     # [4, 1]
    eye16 = np.eye(128, dtype=f16)
    eye32 = np.eye(128, dtype=np.float32)

    xall = bank_out.reshape(NB, B * L, 2 * D)
    shared = dict(pk=pkarr, okk=okarr, w1t=w1tb, b1c=b1cb, w2t=w2tb, b2c=b2cb,
                  eye16=eye16, eye32=eye32)
    in_maps = []
    for k in range(NCORES):
        xin = np.ascontiguousarray(xall[:, k * ROWS:(k + 1) * ROWS, :])
        in_maps.append(dict(shared, xin=xin))
    return in_maps


_nc_cache = {}


def kernel(**inputs):
    from concourse.bass_utils import run_bass_kernel_spmd

    if "nc" not in _nc_cache:
        _nc_cache["nc"] = build_program(gelu_exact=True)
    nc = _nc_cache["nc"]
    in_maps = host_prep(inputs)
    res = run_bass_kernel_spmd(nc, in_maps, core_ids=list(range(NCORES)))
    out = np.concatenate([r["out"] for r in res.results], axis=0)
    return np.ascontiguousarray(out.reshape(B, L, D, 2))
